# revision 10
# baseline (speedup 1.0000x reference)
"""LoRA linear (dropout -> x @ A.T @ B.T * scaling) on 8 TRN2 NeuronCores.

Data-parallel over tokens: each core handles T/8 = 2048 tokens; lora_A/lora_B
are replicated.

Precision plan: all device traffic is bf16 (48MB/core instead of 96MB fp32),
and the PE runs at 1 cycle/row instead of fp32's 4. The dropout compare stays
EXACT on device: the host ships u16 = bf16(drop_u - 0.1); bf16 shares fp32's
exponent range and the fp32 subtraction is exact near 0.1 (Sterbenz), so
sign(u16) == sign(drop_u - 0.1) bit-for-bit and the device mask is
(u16 >= 0). The 1/(1-p) and alpha/r scalings are folded into lora_B on host.

Layout plan: the host packs x and u into the exact transposed SBUF tile
layout ([i-chunk on partitions, tokens free]), so no on-chip transpose is
needed and loads are one 2MB + one 1MB DMA per block with 16KB/8KB
contiguous per partition. Per 256-token block:
  DMA x,u8 -> DVE mask=(u8>=0), xd=x*mask in 4 column chunks -> 32
  accumulating matmuls hT[64,256] (interleaved with DVE chunks) -> per
  128-token half: 8 matmuls out[128,512] + ACT cast-copy + 1MB store.
Host upcasts the bf16 output to fp32.
"""

import sys

sys.path.insert(0, "/opt/trn_rl_repo")

import ml_dtypes
import numpy as np

import concourse.bacc as bacc
import concourse.tile as tile
from concourse import mybir
from concourse.bass_utils import run_bass_kernel_spmd

N_CORES = 8
T, IN, OUT, R = 16384, 4096, 4096, 64
TS = T // N_CORES  # tokens per core (2048)
P_DROP = 0.1
SCALE = (128.0 / 64.0) / (1.0 - P_DROP)  # alpha/r * 1/(1-p), folded into B

F32 = mybir.dt.float32
BF16 = mybir.dt.bfloat16
FP8 = mybir.dt.float8e5
NPBF16 = np.dtype(ml_dtypes.bfloat16)
NPFP8 = np.dtype(ml_dtypes.float8_e5m2)

TB = 256  # tokens per block
NB = TS // TB  # blocks per core (8)
KC = IN // 128  # contraction chunks (32)
W = KC * TB  # row width of one packed tensor (8192)
NCH = 2  # DVE column chunks per block
CH = W // NCH  # chunk width (2048)
KCH = KC // NCH  # contraction chunks per DVE chunk (8)


def _emit(tc, x, u, a, b, o):
    """Per-core program. x/u are [NB*128, W] packed transposed blocks with
    col = kc*TB + t -> x[blk*TB+t, kc*128+p]. a is [128, KC*64] packed A
    chunks, b is [64, OUT] scaled B transposed, o is [TS, OUT] natural."""
    nc = tc.nc
    from contextlib import ExitStack

    with ExitStack() as ctx:
        const = ctx.enter_context(tc.tile_pool(name="const", bufs=1))
        xpool = ctx.enter_context(tc.tile_pool(name="xp", bufs=3))
        upool = ctx.enter_context(tc.tile_pool(name="up", bufs=3))
        mpool = ctx.enter_context(tc.tile_pool(name="mp", bufs=2))
        hpool = ctx.enter_context(tc.tile_pool(name="hp", bufs=2))
        opool = ctx.enter_context(tc.tile_pool(name="op", bufs=3))
        psh = ctx.enter_context(tc.tile_pool(name="psh", bufs=2, space="PSUM"))
        pso = ctx.enter_context(tc.tile_pool(name="pso", bufs=3, space="PSUM"))

        a_sb = const.tile([128, KC * R], BF16)
        nc.scalar.dma_start(a_sb[:], a[:, :])
        b_sb = const.tile([R, OUT], BF16)
        nc.scalar.dma_start(b_sb[:], b[:, :])

        for blk in range(NB):
            rows = slice(blk * 128, (blk + 1) * 128)
            ut = upool.tile([128, W], FP8)
            xt = xpool.tile([128, W], BF16)
            for c in range(NCH):
                cs = slice(c * CH, (c + 1) * CH)
                nc.sync.dma_start(ut[:, cs], u[rows, c * CH : (c + 1) * CH])
                nc.sync.dma_start(xt[:, cs], x[rows, c * CH : (c + 1) * CH])

            ph = psh.tile([R, TB], F32)
            for c in range(NCH):
                cs = slice(c * CH, (c + 1) * CH)
                mt = mpool.tile([128, CH], BF16)
                nc.vector.tensor_scalar(
                    mt[:], ut[:, cs], 0.0, None, mybir.AluOpType.is_ge
                )
                nc.vector.tensor_tensor(
                    xt[:, cs], xt[:, cs], mt[:], mybir.AluOpType.mult
                )
                # hT[64, TB] += a_kc.T @ xdT_kc over this chunk's kcs
                for j in range(KCH):
                    kc = c * KCH + j
                    nc.tensor.matmul(
                        ph[:],
                        a_sb[:, kc * R : (kc + 1) * R],
                        xt[:, kc * TB : (kc + 1) * TB],
                        start=(kc == 0),
                        stop=(kc == KC - 1),
                    )
            hT = hpool.tile([R, TB], BF16)
            nc.scalar.copy(hT[:], ph[:])

            # out[TB, OUT] = hT.T @ b_sb, stored per 128-token half
            for tc2 in range(2):
                osb = opool.tile([128, OUT], BF16)
                for g in range(OUT // 1024):
                    po = pso.tile([128, 1024], F32)
                    for j in range(2):
                        oc = g * 2 + j
                        nc.tensor.matmul(
                            po[:, j * 512 : (j + 1) * 512],
                            hT[:, tc2 * 128 : (tc2 + 1) * 128],
                            b_sb[:, oc * 512 : (oc + 1) * 512],
                            start=True,
                            stop=True,
                        )
                    nc.scalar.copy(
                        osb[:, g * 1024 : (g + 1) * 1024], po[:]
                    )
                nc.scalar.dma_start(
                    o[blk * TB + tc2 * 128 : blk * TB + (tc2 + 1) * 128, :],
                    osb[:],
                )


def build_nc():
    nc = bacc.Bacc()
    x_d = nc.declare_dram_parameter("x", [NB * 128, W], BF16, isOutput=False)
    u_d = nc.declare_dram_parameter("u", [NB * 128, W], FP8, isOutput=False)
    a_d = nc.declare_dram_parameter("a", [128, KC * R], BF16, isOutput=False)
    b_d = nc.declare_dram_parameter("b", [R, OUT], BF16, isOutput=False)
    o_d = nc.declare_dram_parameter("o", [TS, OUT], BF16, isOutput=True)
    with tile.TileContext(nc) as tc:
        _emit(tc, x_d[:], u_d[:], a_d[:], b_d[:], o_d[:])
    if not nc.is_finalized():
        nc.finalize()
    return nc


_NC_CACHE = None


def _get_nc():
    global _NC_CACHE
    if _NC_CACHE is None:
        _NC_CACHE = build_nc()
    return _NC_CACHE


def _pack_tokens(arr, npdt):
    """[T, IN] fp32 -> per-core [NB*128, W] in transposed block layout:
    out[c][blk*128+p, kc*TB+t] = arr[c*TS + blk*TB + t, kc*128+p]."""
    a5 = arr.reshape(N_CORES, NB, TB, KC, 128).transpose(0, 1, 4, 3, 2)
    return np.ascontiguousarray(a5.astype(npdt)).reshape(
        N_CORES, NB * 128, W
    )


def _in_maps(x, lora_A, lora_B, drop_u):
    xp = _pack_tokens(np.asarray(x, dtype=np.float32), NPBF16)
    up = _pack_tokens(
        np.asarray(drop_u, dtype=np.float32) - np.float32(P_DROP), NPFP8
    )
    # a[p, kc*64+r] = A[r, kc*128+p]
    ap = np.ascontiguousarray(
        np.asarray(lora_A, dtype=np.float32)
        .T.reshape(KC, 128, R)
        .transpose(1, 0, 2)
        .astype(NPBF16)
    ).reshape(128, KC * R)
    bp = np.ascontiguousarray(
        (np.asarray(lora_B, dtype=np.float32) * np.float32(SCALE))
        .T.astype(NPBF16)
    )
    return [
        {"x": xp[c], "u": up[c], "a": ap, "b": bp} for c in range(N_CORES)
    ]


def run_spmd(x, lora_A, lora_B, drop_u, **kw):
    res = run_bass_kernel_spmd(
        _get_nc(), _in_maps(x, lora_A, lora_B, drop_u), list(range(N_CORES)), **kw
    )
    out = np.concatenate(
        [np.asarray(r["o"]).astype(np.float32) for r in res.results], axis=0
    )
    return out, res


def kernel(x, lora_A, lora_B, drop_u):
    out, _ = run_spmd(x, lora_A, lora_B, drop_u)
    return out


# revision 11
# speedup vs baseline: 1.0129x; 1.0129x over previous
"""LoRA linear (dropout -> x @ A.T @ B.T * scaling) on 8 TRN2 NeuronCores.

Data-parallel over tokens: each core handles T/8 = 2048 tokens; lora_A/lora_B
are replicated.

Precision plan: all device traffic is bf16 (48MB/core instead of 96MB fp32),
and the PE runs at 1 cycle/row instead of fp32's 4. The dropout compare stays
EXACT on device: the host ships u16 = bf16(drop_u - 0.1); bf16 shares fp32's
exponent range and the fp32 subtraction is exact near 0.1 (Sterbenz), so
sign(u16) == sign(drop_u - 0.1) bit-for-bit and the device mask is
(u16 >= 0). The 1/(1-p) and alpha/r scalings are folded into lora_B on host.

Layout plan: the host packs x and u into the exact transposed SBUF tile
layout ([i-chunk on partitions, tokens free]), so no on-chip transpose is
needed and loads are one 2MB + one 1MB DMA per block with 16KB/8KB
contiguous per partition. Per 256-token block:
  DMA x,u8 -> DVE mask=(u8>=0), xd=x*mask in 4 column chunks -> 32
  accumulating matmuls hT[64,256] (interleaved with DVE chunks) -> per
  128-token half: 8 matmuls out[128,512] + ACT cast-copy + 1MB store.
Host upcasts the bf16 output to fp32.
"""

import sys

sys.path.insert(0, "/opt/trn_rl_repo")

import ml_dtypes
import numpy as np

import concourse.bacc as bacc
import concourse.tile as tile
from concourse import mybir
from concourse.bass_utils import run_bass_kernel_spmd

N_CORES = 8
T, IN, OUT, R = 16384, 4096, 4096, 64
TS = T // N_CORES  # tokens per core (2048)
P_DROP = 0.1
SCALE = (128.0 / 64.0) / (1.0 - P_DROP)  # alpha/r * 1/(1-p), folded into B

F32 = mybir.dt.float32
BF16 = mybir.dt.bfloat16
FP8 = mybir.dt.float8e5
NPBF16 = np.dtype(ml_dtypes.bfloat16)
NPFP8 = np.dtype(ml_dtypes.float8_e5m2)

TB = 256  # tokens per block
NB = TS // TB  # blocks per core (8)
KC = IN // 128  # contraction chunks (32)
W = KC * TB  # row width of one packed tensor (8192)
NCH = 2  # DVE column chunks per block
CH = W // NCH  # chunk width (2048)
KCH = KC // NCH  # contraction chunks per DVE chunk (8)


def _emit(tc, x, u, a, b, o):
    """Per-core program. x/u are [NB*128, W] packed transposed blocks with
    col = kc*TB + t -> x[blk*TB+t, kc*128+p]. a is [128, KC*64] packed A
    chunks, b is [64, OUT] scaled B transposed, o is [TS, OUT] natural."""
    nc = tc.nc
    from contextlib import ExitStack

    with ExitStack() as ctx:
        const = ctx.enter_context(tc.tile_pool(name="const", bufs=1))
        xpool = ctx.enter_context(tc.tile_pool(name="xp", bufs=3))
        upool = ctx.enter_context(tc.tile_pool(name="up", bufs=3))
        mpool = ctx.enter_context(tc.tile_pool(name="mp", bufs=2))
        hpool = ctx.enter_context(tc.tile_pool(name="hp", bufs=2))
        opool = ctx.enter_context(tc.tile_pool(name="op", bufs=3))
        psh = ctx.enter_context(tc.tile_pool(name="psh", bufs=2, space="PSUM"))
        pso = ctx.enter_context(tc.tile_pool(name="pso", bufs=3, space="PSUM"))

        a_sb = const.tile([128, KC * R], BF16)
        nc.scalar.dma_start(a_sb[:], a[:, :])
        b_sb = const.tile([R, OUT], BF16)
        nc.scalar.dma_start(b_sb[:], b[:, :])

        for blk in range(NB):
            rows = slice(blk * 128, (blk + 1) * 128)
            ut = upool.tile([128, W], FP8)
            xt = xpool.tile([128, W], BF16)
            if blk == 0:
                # split the first block's loads so the pipeline starts early
                for c in range(NCH):
                    cs = slice(c * CH, (c + 1) * CH)
                    nc.sync.dma_start(ut[:, cs], u[rows, c * CH : (c + 1) * CH])
                    nc.sync.dma_start(xt[:, cs], x[rows, c * CH : (c + 1) * CH])
            else:
                nc.sync.dma_start(ut[:], u[rows, :])
                nc.sync.dma_start(xt[:], x[rows, :])

            ph = psh.tile([R, TB], F32)
            for c in range(NCH):
                cs = slice(c * CH, (c + 1) * CH)
                mt = mpool.tile([128, CH], BF16)
                nc.vector.tensor_scalar(
                    mt[:], ut[:, cs], 0.0, None, mybir.AluOpType.is_ge
                )
                nc.vector.tensor_tensor(
                    xt[:, cs], xt[:, cs], mt[:], mybir.AluOpType.mult
                )
                # hT[64, TB] += a_kc.T @ xdT_kc over this chunk's kcs
                for j in range(KCH):
                    kc = c * KCH + j
                    nc.tensor.matmul(
                        ph[:],
                        a_sb[:, kc * R : (kc + 1) * R],
                        xt[:, kc * TB : (kc + 1) * TB],
                        start=(kc == 0),
                        stop=(kc == KC - 1),
                    )
            hT = hpool.tile([R, TB], BF16)
            nc.scalar.copy(hT[:], ph[:])

            # out[TB, OUT] = hT.T @ b_sb, stored per 128-token half
            for tc2 in range(2):
                osb = opool.tile([128, OUT], BF16)
                for g in range(OUT // 1024):
                    po = pso.tile([128, 1024], F32)
                    for j in range(2):
                        oc = g * 2 + j
                        nc.tensor.matmul(
                            po[:, j * 512 : (j + 1) * 512],
                            hT[:, tc2 * 128 : (tc2 + 1) * 128],
                            b_sb[:, oc * 512 : (oc + 1) * 512],
                            start=True,
                            stop=True,
                        )
                    nc.scalar.copy(
                        osb[:, g * 1024 : (g + 1) * 1024], po[:]
                    )
                nc.scalar.dma_start(
                    o[blk * TB + tc2 * 128 : blk * TB + (tc2 + 1) * 128, :],
                    osb[:],
                )


def build_nc():
    nc = bacc.Bacc()
    x_d = nc.declare_dram_parameter("x", [NB * 128, W], BF16, isOutput=False)
    u_d = nc.declare_dram_parameter("u", [NB * 128, W], FP8, isOutput=False)
    a_d = nc.declare_dram_parameter("a", [128, KC * R], BF16, isOutput=False)
    b_d = nc.declare_dram_parameter("b", [R, OUT], BF16, isOutput=False)
    o_d = nc.declare_dram_parameter("o", [TS, OUT], BF16, isOutput=True)
    with tile.TileContext(nc) as tc:
        _emit(tc, x_d[:], u_d[:], a_d[:], b_d[:], o_d[:])
    if not nc.is_finalized():
        nc.finalize()
    return nc


_NC_CACHE = None


def _get_nc():
    global _NC_CACHE
    if _NC_CACHE is None:
        _NC_CACHE = build_nc()
    return _NC_CACHE


def _pack_tokens(arr, npdt):
    """[T, IN] fp32 -> per-core [NB*128, W] in transposed block layout:
    out[c][blk*128+p, kc*TB+t] = arr[c*TS + blk*TB + t, kc*128+p]."""
    a5 = arr.reshape(N_CORES, NB, TB, KC, 128).transpose(0, 1, 4, 3, 2)
    return np.ascontiguousarray(a5.astype(npdt)).reshape(
        N_CORES, NB * 128, W
    )


def _in_maps(x, lora_A, lora_B, drop_u):
    xp = _pack_tokens(np.asarray(x, dtype=np.float32), NPBF16)
    up = _pack_tokens(
        np.asarray(drop_u, dtype=np.float32) - np.float32(P_DROP), NPFP8
    )
    # a[p, kc*64+r] = A[r, kc*128+p]
    ap = np.ascontiguousarray(
        np.asarray(lora_A, dtype=np.float32)
        .T.reshape(KC, 128, R)
        .transpose(1, 0, 2)
        .astype(NPBF16)
    ).reshape(128, KC * R)
    bp = np.ascontiguousarray(
        (np.asarray(lora_B, dtype=np.float32) * np.float32(SCALE))
        .T.astype(NPBF16)
    )
    return [
        {"x": xp[c], "u": up[c], "a": ap, "b": bp} for c in range(N_CORES)
    ]


def run_spmd(x, lora_A, lora_B, drop_u, **kw):
    res = run_bass_kernel_spmd(
        _get_nc(), _in_maps(x, lora_A, lora_B, drop_u), list(range(N_CORES)), **kw
    )
    out = np.concatenate(
        [np.asarray(r["o"]).astype(np.float32) for r in res.results], axis=0
    )
    return out, res


def kernel(x, lora_A, lora_B, drop_u):
    out, _ = run_spmd(x, lora_A, lora_B, drop_u)
    return out


# revision 12
# speedup vs baseline: 1.0477x; 1.0343x over previous
"""LoRA linear (dropout -> x @ A.T @ B.T * scaling) on 8 TRN2 NeuronCores.

Data-parallel over tokens: each core handles T/8 = 2048 tokens; lora_A/lora_B
are replicated.

Precision plan: all device traffic is bf16 (48MB/core instead of 96MB fp32),
and the PE runs at 1 cycle/row instead of fp32's 4. The dropout compare stays
EXACT on device: the host ships u16 = bf16(drop_u - 0.1); bf16 shares fp32's
exponent range and the fp32 subtraction is exact near 0.1 (Sterbenz), so
sign(u16) == sign(drop_u - 0.1) bit-for-bit and the device mask is
(u16 >= 0). The 1/(1-p) and alpha/r scalings are folded into lora_B on host.

Layout plan: the host packs x and u into the exact transposed SBUF tile
layout ([i-chunk on partitions, tokens free]), so no on-chip transpose is
needed and loads are one 2MB + one 1MB DMA per block with 16KB/8KB
contiguous per partition. Per 256-token block:
  DMA x,u8 -> DVE mask=(u8>=0), xd=x*mask in 4 column chunks -> 32
  accumulating matmuls hT[64,256] (interleaved with DVE chunks) -> per
  128-token half: 8 matmuls out[128,512] + ACT cast-copy + 1MB store.
Host upcasts the bf16 output to fp32.
"""

import sys

sys.path.insert(0, "/opt/trn_rl_repo")

import ml_dtypes
import numpy as np

import concourse.bacc as bacc
import concourse.tile as tile
from concourse import mybir
from concourse.bass_utils import run_bass_kernel_spmd

N_CORES = 8
T, IN, OUT, R = 16384, 4096, 4096, 64
TS = T // N_CORES  # tokens per core (2048)
P_DROP = 0.1
SCALE = (128.0 / 64.0) / (1.0 - P_DROP)  # alpha/r * 1/(1-p), folded into B

F32 = mybir.dt.float32
BF16 = mybir.dt.bfloat16
FP8 = mybir.dt.float8e5
NPBF16 = np.dtype(ml_dtypes.bfloat16)
NPFP8 = np.dtype(ml_dtypes.float8_e5m2)

TB = 256  # tokens per block
NB = TS // TB  # blocks per core (8)
KC = IN // 128  # contraction chunks (32)
W = KC * TB  # row width of one packed tensor (8192)
NCH = 2  # DVE column chunks per block
CH = W // NCH  # chunk width (2048)
KCH = KC // NCH  # contraction chunks per DVE chunk (8)


def _emit(tc, x, u, a, b, o):
    """Per-core program. x/u are [NB*128, W] packed transposed blocks with
    col = kc*TB + t -> x[blk*TB+t, kc*128+p]. a is [128, KC*64] packed A
    chunks, b is [64, OUT] scaled B transposed, o is [TS, OUT] natural."""
    nc = tc.nc
    from contextlib import ExitStack

    with ExitStack() as ctx:
        const = ctx.enter_context(tc.tile_pool(name="const", bufs=1))
        xpool = ctx.enter_context(tc.tile_pool(name="xp", bufs=3))
        upool = ctx.enter_context(tc.tile_pool(name="up", bufs=3))
        mpool = ctx.enter_context(tc.tile_pool(name="mp", bufs=2))
        hpool = ctx.enter_context(tc.tile_pool(name="hp", bufs=2))
        opool = ctx.enter_context(tc.tile_pool(name="op", bufs=3))
        psh = ctx.enter_context(tc.tile_pool(name="psh", bufs=2, space="PSUM"))
        pso = ctx.enter_context(tc.tile_pool(name="pso", bufs=3, space="PSUM"))

        a_sb = const.tile([128, KC * R], BF16)
        nc.scalar.dma_start(a_sb[:], a[:, :])
        b_sb = const.tile([R, OUT], BF16)
        nc.scalar.dma_start(b_sb[:], b[:, :])

        for blk in range(NB):
            rows = slice(blk * 128, (blk + 1) * 128)
            ut = upool.tile([128, W], FP8)
            nc.sync.dma_start(ut[:], u[rows, :])
            xt = xpool.tile([128, W], BF16)
            nc.sync.dma_start(xt[:], x[rows, :])

            ph = psh.tile([R, TB], F32)
            for c in range(NCH):
                cs = slice(c * CH, (c + 1) * CH)
                mt = mpool.tile([128, CH], BF16)
                nc.vector.tensor_scalar(
                    mt[:], ut[:, cs], 0.0, None, mybir.AluOpType.is_ge
                )
                nc.vector.tensor_tensor(
                    xt[:, cs], xt[:, cs], mt[:], mybir.AluOpType.mult
                )
                # hT[64, TB] += a_kc.T @ xdT_kc over this chunk's kcs
                for j in range(KCH):
                    kc = c * KCH + j
                    nc.tensor.matmul(
                        ph[:],
                        a_sb[:, kc * R : (kc + 1) * R],
                        xt[:, kc * TB : (kc + 1) * TB],
                        start=(kc == 0),
                        stop=(kc == KC - 1),
                    )
            hT = hpool.tile([R, TB], BF16)
            nc.scalar.copy(hT[:], ph[:])

            # out[TB, OUT] = hT.T @ b_sb, stored per 128-token half
            for tc2 in range(2):
                osb = opool.tile([128, OUT], BF16)
                for g in range(OUT // 1024):
                    po = pso.tile([128, 1024], F32)
                    for j in range(2):
                        oc = g * 2 + j
                        nc.tensor.matmul(
                            po[:, j * 512 : (j + 1) * 512],
                            hT[:, tc2 * 128 : (tc2 + 1) * 128],
                            b_sb[:, oc * 512 : (oc + 1) * 512],
                            start=True,
                            stop=True,
                        )
                    nc.scalar.copy(
                        osb[:, g * 1024 : (g + 1) * 1024], po[:]
                    )
                nc.scalar.dma_start(
                    o[blk * TB + tc2 * 128 : blk * TB + (tc2 + 1) * 128, :],
                    osb[:],
                )


def build_nc():
    nc = bacc.Bacc()
    x_d = nc.declare_dram_parameter("x", [NB * 128, W], BF16, isOutput=False)
    u_d = nc.declare_dram_parameter("u", [NB * 128, W], FP8, isOutput=False)
    a_d = nc.declare_dram_parameter("a", [128, KC * R], BF16, isOutput=False)
    b_d = nc.declare_dram_parameter("b", [R, OUT], BF16, isOutput=False)
    o_d = nc.declare_dram_parameter("o", [TS, OUT], BF16, isOutput=True)
    with tile.TileContext(nc) as tc:
        _emit(tc, x_d[:], u_d[:], a_d[:], b_d[:], o_d[:])
    if not nc.is_finalized():
        nc.finalize()
    return nc


_NC_CACHE = None


def _get_nc():
    global _NC_CACHE
    if _NC_CACHE is None:
        _NC_CACHE = build_nc()
    return _NC_CACHE


def _pack_tokens(arr, npdt):
    """[T, IN] fp32 -> per-core [NB*128, W] in transposed block layout:
    out[c][blk*128+p, kc*TB+t] = arr[c*TS + blk*TB + t, kc*128+p]."""
    a5 = arr.reshape(N_CORES, NB, TB, KC, 128).transpose(0, 1, 4, 3, 2)
    return np.ascontiguousarray(a5.astype(npdt)).reshape(
        N_CORES, NB * 128, W
    )


def _in_maps(x, lora_A, lora_B, drop_u):
    xp = _pack_tokens(np.asarray(x, dtype=np.float32), NPBF16)
    up = _pack_tokens(
        np.asarray(drop_u, dtype=np.float32) - np.float32(P_DROP), NPFP8
    )
    # a[p, kc*64+r] = A[r, kc*128+p]
    ap = np.ascontiguousarray(
        np.asarray(lora_A, dtype=np.float32)
        .T.reshape(KC, 128, R)
        .transpose(1, 0, 2)
        .astype(NPBF16)
    ).reshape(128, KC * R)
    bp = np.ascontiguousarray(
        (np.asarray(lora_B, dtype=np.float32) * np.float32(SCALE))
        .T.astype(NPBF16)
    )
    return [
        {"x": xp[c], "u": up[c], "a": ap, "b": bp} for c in range(N_CORES)
    ]


def run_spmd(x, lora_A, lora_B, drop_u, **kw):
    res = run_bass_kernel_spmd(
        _get_nc(), _in_maps(x, lora_A, lora_B, drop_u), list(range(N_CORES)), **kw
    )
    out = np.concatenate(
        [np.asarray(r["o"]).astype(np.float32) for r in res.results], axis=0
    )
    return out, res


def kernel(x, lora_A, lora_B, drop_u):
    out, _ = run_spmd(x, lora_A, lora_B, drop_u)
    return out
